# revision 1
# baseline (speedup 1.0000x reference)
"""KKT loss kernel for Trainium2, 8 NeuronCores.

Sharding: batch axis — core c handles LP instances [8c, 8c+8).

Host side (this file, numpy): index preprocessing. COO entries are
routed into a padded-ELL layout keyed by (row mod 128) lanes so the
device-side segment sum becomes a regular windowed reduction:
  pass A (Ax):    entry e -> lane p = row%128, window s = item*32 + row//128,
                  slot = position-within-row; arrays hold a_vals and the
                  gathered x_hat[col] values.
  pass B (AtLam): same with cols: lane = col%128, window = item*64 + col//128,
                  values a_vals and gathered lam_hat[row].
Padding slots carry 0.0 so they contribute nothing to any sum.

Device side (Bass/Tile, per core): y = vals * gathered (DVE), windowed
tensor_reduce -> Ax [128, 256] / AtLam [128, 512], then all loss terms
(relu/square/sums) and a final ones-matmul partition reduction to one
scalar per core. Host sums the 8 per-core scalars (the "all-reduce").
"""
import sys

sys.path.insert(0, "/opt/trn_rl_repo")

from contextlib import ExitStack

import numpy as np

from concourse import bacc, mybir, tile
from concourse.bass_utils import run_bass_kernel_spmd

B = 64
M = 4096
N = 8192
IPC = 8          # items per core
NCORES = 8
W_PRIMAL, W_DUAL, W_STAT, W_COMP = 0.1, 0.1, 0.6, 0.2

_cache = {}


def _build(KA, KAL, KB, KBL):
    """Per-core Bass program; both passes use two window classes (hi, lo)."""
    key = (KA, KAL, KB, KBL)
    if key in _cache:
        return _cache[key]

    LAH = 8 * KA           # pass A high-degree class (ranks < 1024)
    LAL = 24 * KAL         # pass A low-degree class
    LA = LAH + LAL         # per-item flat size, pass A
    LBH = 16 * KB          # pass B high-degree class (ranks < 2048)
    LBL = 48 * KBL         # pass B low-degree class
    LB = LBH + LBL         # per-item flat size, pass B
    CH = 2         # items per stream chunk
    f32 = mybir.dt.float32
    bf16 = mybir.dt.bfloat16
    add = mybir.AluOpType.add
    mult = mybir.AluOpType.mult

    nc = bacc.Bacc("TRN2", target_bir_lowering=False, debug=False,
                   num_devices=NCORES)

    xgA = nc.dram_tensor("xgA", [128, IPC, LA], bf16, kind="ExternalInput").ap()
    vaA = nc.dram_tensor("vaA", [128, IPC, LA], bf16, kind="ExternalInput").ap()
    lgB = nc.dram_tensor("lgB", [128, IPC, LB], bf16, kind="ExternalInput").ap()
    vaB = nc.dram_tensor("vaB", [128, IPC, LB], bf16, kind="ExternalInput").ap()
    b_l = nc.dram_tensor("b_l", [128, 256], f32, kind="ExternalInput").ap()
    lam_l = nc.dram_tensor("lam_l", [128, 256], f32, kind="ExternalInput").ap()
    c_l = nc.dram_tensor("c_l", [128, 512], f32, kind="ExternalInput").ap()
    x_l = nc.dram_tensor("x_l", [128, 512], f32, kind="ExternalInput").ap()
    loss_d = nc.dram_tensor("loss", [1, 1], f32, kind="ExternalOutput").ap()

    with tile.TileContext(nc) as tc:
        with (
            tc.tile_pool(name="stream", bufs=3) as sp,
            tc.tile_pool(name="persist", bufs=1) as pp,
            tc.tile_pool(name="psum", bufs=1, space="PSUM") as qp,
            ExitStack() as ctx,
        ):
            ax = pp.tile([128, 256], f32)       # Ax per (lane, item*32+s)
            at = pp.tile([128, 512], f32)       # AtLam per (lane, item*64+s)

            # small first chunk so DVE starts sooner (pipeline fill)
            ax4 = ax[:].rearrange("p (it s) -> p it s", s=32)
            chunksA = [(0, 1), (1, 3), (3, 5), (5, 7), (7, 8)]
            for i0, i1 in chunksA:
                n = i1 - i0
                for coff, clen, Kc, soff, scnt in (
                        (0, LAH, KA, 0, 8), (LAH, LAL, KAL, 8, 24)):
                    xg_t = sp.tile([128, n * clen], bf16, tag="g")
                    va_t = sp.tile([128, n * clen], bf16, tag="v")
                    nc.sync.dma_start(xg_t[:],
                                      xgA[:, i0:i1, coff:coff + clen])
                    nc.sync.dma_start(va_t[:],
                                      vaA[:, i0:i1, coff:coff + clen])
                    nc.vector.tensor_mul(xg_t[:], xg_t[:], va_t[:])
                    nc.vector.tensor_reduce(
                        ax4[:, i0:i1, soff:soff + scnt],
                        xg_t[:].rearrange("p (it s k) -> p it s k",
                                          it=n, k=Kc),
                        axis=mybir.AxisListType.X, op=add)

            at4 = at[:].rearrange("p (it s) -> p it s", s=64)
            for i in range(0, IPC, CH):
                for coff, clen, Kc, soff, scnt in (
                        (0, LBH, KB, 0, 16), (LBH, LBL, KBL, 16, 48)):
                    lg_t = sp.tile([128, CH * clen], bf16, tag="g")
                    vb_t = sp.tile([128, CH * clen], bf16, tag="v")
                    nc.sync.dma_start(lg_t[:],
                                      lgB[:, i:i + CH, coff:coff + clen])
                    nc.sync.dma_start(vb_t[:],
                                      vaB[:, i:i + CH, coff:coff + clen])
                    nc.vector.tensor_mul(lg_t[:], lg_t[:], vb_t[:])
                    nc.vector.tensor_reduce(
                        at4[:, i:i + CH, soff:soff + scnt],
                        lg_t[:].rearrange("p (it s k) -> p it s k",
                                          it=CH, k=Kc),
                        axis=mybir.AxisListType.X, op=add)

            # epilogue
            bt = pp.tile([128, 256], f32)
            lt = pp.tile([128, 256], f32)
            ct = pp.tile([128, 512], f32)
            xt = pp.tile([128, 512], f32)
            nc.sync.dma_start(bt[:], b_l)
            nc.sync.dma_start(lt[:], lam_l)
            nc.sync.dma_start(ct[:], c_l)
            nc.sync.dma_start(xt[:], x_l)

            g1 = pp.tile([128, 1], f32)
            g2 = pp.tile([128, 1], f32)
            g3 = pp.tile([128, 1], f32)

            axmb = pp.tile([128, 256], f32)
            nc.vector.tensor_sub(axmb[:], ax[:], bt[:])
            zt = pp.tile([128, 512], f32)
            nc.vector.tensor_add(zt[:], ct[:], at[:])

            # group terms by weight, square into shared scratch, one
            # reduce per weight group
            sc1 = pp.tile([128, 1024], f32)   # relu(axmb)^2|min(lam,0)^2|min(x,0)^2
            sc2 = pp.tile([128, 768], f32)    # (lam*axmb)^2|(relu(z)*x)^2
            sc3 = pp.tile([128, 512], f32)    # min(z,0)^2
            t2 = pp.tile([128, 256], f32)
            t5 = pp.tile([128, 512], f32)

            nc.vector.tensor_scalar_max(t2[:], axmb[:], 0.0)
            nc.vector.tensor_mul(sc1[:, 0:256], t2[:], t2[:])
            nc.vector.tensor_scalar_min(t2[:], lt[:], 0.0)
            nc.vector.tensor_mul(sc1[:, 256:512], t2[:], t2[:])
            nc.vector.tensor_scalar_min(t5[:], xt[:], 0.0)
            nc.vector.tensor_mul(sc1[:, 512:1024], t5[:], t5[:])

            nc.vector.tensor_mul(t2[:], lt[:], axmb[:])
            nc.vector.tensor_mul(sc2[:, 0:256], t2[:], t2[:])
            nc.vector.tensor_scalar_max(t5[:], zt[:], 0.0)
            nc.vector.tensor_mul(t5[:], t5[:], xt[:])
            nc.vector.tensor_mul(sc2[:, 256:768], t5[:], t5[:])

            nc.vector.tensor_scalar_min(t5[:], zt[:], 0.0)
            nc.vector.tensor_mul(sc3[:], t5[:], t5[:])

            nc.vector.tensor_reduce(g1[:], sc1[:],
                                    axis=mybir.AxisListType.X, op=add)
            nc.vector.tensor_reduce(g2[:], sc2[:],
                                    axis=mybir.AxisListType.X, op=add)
            nc.vector.tensor_reduce(g3[:], sc3[:],
                                    axis=mybir.AxisListType.X, op=add)

            # partial = c_mn*g1 + c_cp*g2 + c_st*g3
            c_mn = 1.0 / (float(M + N) * float(B)) * W_PRIMAL  # = W_DUAL
            c_cp = 1.0 / (float(M + N) * float(B)) * W_COMP
            c_st = 1.0 / (float(N) * float(B)) * W_STAT
            part = pp.tile([128, 1], f32)
            nc.vector.tensor_scalar_mul(part[:], g1[:], c_mn)
            nc.vector.scalar_tensor_tensor(
                part[:], g2[:], c_cp, part[:], op0=mult, op1=add)
            nc.vector.scalar_tensor_tensor(
                part[:], g3[:], c_st, part[:], op0=mult, op1=add)

            ones = pp.tile([128, 1], f32)
            nc.vector.memset(ones[:], 1.0)
            ps = qp.tile([1, 1], f32)
            nc.tensor.matmul(ps[:], lhsT=part[:], rhs=ones[:],
                             start=True, stop=True)
            loss_sb = pp.tile([1, 1], f32)
            nc.vector.tensor_copy(loss_sb[:], ps[:])
            nc.sync.dma_start(loss_d, loss_sb[:])

    nc.compile()
    _cache[key] = nc
    return nc


def _prep(x_hat, lam_hat, a_vals, a_rows, a_cols, b_pad, c_pad):
    """Build per-core input dicts (all numpy, index work + gathers only)."""
    nnz = a_vals.shape[0]
    RPC = IPC * M    # rows per core
    CPC = IPC * N    # cols per core

    deg_r = np.bincount(a_rows, minlength=B * M)
    deg_c = np.bincount(a_cols, minlength=B * N)
    KA = max(2, (int(deg_r.max()) + 1) & ~1)
    KB = max(2, (int(deg_c.max()) + 1) & ~1)
    degr2 = deg_r.reshape(B, M)
    order_desc_r = np.argsort(-degr2, axis=1, kind="stable")
    rank_of_r = np.empty_like(order_desc_r)
    np.put_along_axis(rank_of_r, order_desc_r,
                      np.broadcast_to(np.arange(M, dtype=order_desc_r.dtype),
                                      (B, M)), axis=1)
    degr_ranked = np.take_along_axis(degr2, order_desc_r, axis=1)
    KAL = max(2, (int(degr_ranked[:, 1024:].max()) + 1) & ~1)
    # rank cols of each item by degree (desc); ranks >= 2048 use a small
    # window KBL.  Loss sums are order-invariant, so columns may be
    # permuted as long as c_l / x_l use the same permutation.
    degc2 = deg_c.reshape(B, N)
    order_desc = np.argsort(-degc2, axis=1, kind="stable")  # rank -> col
    rank_of = np.empty_like(order_desc)
    np.put_along_axis(rank_of, order_desc,
                      np.broadcast_to(np.arange(N, dtype=order_desc.dtype),
                                      (B, N)), axis=1)
    deg_ranked = np.take_along_axis(degc2, order_desc, axis=1)
    KBL = max(2, (int(deg_ranked[:, 2048:].max()) + 1) & ~1)
    LA = 8 * KA + 24 * KAL
    LB = 16 * KB + 48 * KBL

    # position of each entry within its row / col
    order_r = np.argsort(a_rows, kind="stable")
    pos_r = np.empty(nnz, np.int64)
    starts = np.zeros(B * M, np.int64)
    np.cumsum(deg_r[:-1], out=starts[1:])
    pos_r[order_r] = np.arange(nnz, dtype=np.int64) - starts[a_rows[order_r]]

    order_c = np.argsort(a_cols, kind="stable")
    pos_c = np.empty(nnz, np.int64)
    starts_c = np.zeros(B * N, np.int64)
    np.cumsum(deg_c[:-1], out=starts_c[1:])
    pos_c[order_c] = np.arange(nnz, dtype=np.int64) - starts_c[a_cols[order_c]]

    rows = a_rows.astype(np.int64)
    cols = a_cols.astype(np.int64)

    # pass A (rank-permuted, two classes)
    core_r = rows // RPC
    item_gr = rows // M
    rankr = rank_of_r[item_gr, rows % M]
    p_r = rankr % 128
    s_lr = rankr // 128
    innerA = np.where(s_lr < 8,
                      s_lr * KA,
                      8 * KA + (s_lr - 8) * KAL) + pos_r
    flatA = ((core_r * 128 + p_r) * IPC + item_gr % IPC) * LA + innerA
    arr_vaA = np.zeros(NCORES * 128 * IPC * LA, np.float32)
    arr_xgA = np.zeros(NCORES * 128 * IPC * LA, np.float32)
    arr_vaA[flatA] = a_vals
    arr_xgA[flatA] = x_hat[a_cols]
    arr_vaA = arr_vaA.reshape(NCORES, 128, IPC, LA)
    arr_xgA = arr_xgA.reshape(NCORES, 128, IPC, LA)

    # pass B (rank-permuted, two classes)
    core_c = cols // CPC
    item_g = cols // N
    rank = rank_of[item_g, cols % N]
    p_c = rank % 128
    s_local = rank // 128
    inner = np.where(s_local < 16,
                     s_local * KB,
                     16 * KB + (s_local - 16) * KBL) + pos_c
    flatB = ((core_c * 128 + p_c) * IPC + item_g % IPC) * LB + inner
    arr_vaB = np.zeros(NCORES * 128 * IPC * LB, np.float32)
    arr_lgB = np.zeros(NCORES * 128 * IPC * LB, np.float32)
    arr_vaB[flatB] = a_vals
    arr_lgB[flatB] = lam_hat[a_rows]
    arr_vaB = arr_vaB.reshape(NCORES, 128, IPC, LB)
    arr_lgB = arr_lgB.reshape(NCORES, 128, IPC, LB)

    # small layouts: [cores, 128, IPC*S]
    b_rank = np.take_along_axis(b_pad.reshape(B, M), order_desc_r, axis=1)
    lam_rank = np.take_along_axis(lam_hat.reshape(B, M), order_desc_r, axis=1)
    b_l = b_rank.reshape(NCORES, IPC, 32, 128).transpose(0, 3, 1, 2) \
        .reshape(NCORES, 128, 256).copy()
    lam_l = lam_rank.reshape(NCORES, IPC, 32, 128).transpose(0, 3, 1, 2) \
        .reshape(NCORES, 128, 256).copy()
    c_rank = np.take_along_axis(c_pad.reshape(B, N), order_desc, axis=1)
    x_rank = np.take_along_axis(x_hat.reshape(B, N), order_desc, axis=1)
    c_l = c_rank.reshape(NCORES, IPC, 64, 128).transpose(0, 3, 1, 2) \
        .reshape(NCORES, 128, 512).copy()
    x_l = x_rank.reshape(NCORES, IPC, 64, 128).transpose(0, 3, 1, 2) \
        .reshape(NCORES, 128, 512).copy()

    import ml_dtypes
    bf16 = ml_dtypes.bfloat16
    in_maps = []
    for c in range(NCORES):
        in_maps.append({
            "xgA": np.ascontiguousarray(arr_xgA[c]).astype(bf16),
            "vaA": np.ascontiguousarray(arr_vaA[c]).astype(bf16),
            "lgB": np.ascontiguousarray(arr_lgB[c]).astype(bf16),
            "vaB": np.ascontiguousarray(arr_vaB[c]).astype(bf16),
            "b_l": b_l[c], "lam_l": lam_l[c],
            "c_l": c_l[c], "x_l": x_l[c],
        })
    return KA, KAL, KB, KBL, in_maps


def kernel(x_hat, lam_hat, a_vals, a_rows, a_cols, b_pad, c_pad,
           _trace=False):
    x_hat = np.asarray(x_hat, np.float32)
    lam_hat = np.asarray(lam_hat, np.float32)
    a_vals = np.asarray(a_vals, np.float32)
    a_rows = np.asarray(a_rows)
    a_cols = np.asarray(a_cols)
    b_pad = np.asarray(b_pad, np.float32)
    c_pad = np.asarray(c_pad, np.float32)

    KA, KAL, KB, KBL, in_maps = _prep(x_hat, lam_hat, a_vals, a_rows,
                                      a_cols, b_pad, c_pad)
    nc = _build(KA, KAL, KB, KBL)
    res = run_bass_kernel_spmd(nc, in_maps, core_ids=list(range(NCORES)),
                               trace=_trace)
    total = np.float32(0.0)
    for c in range(NCORES):
        total += res.results[c]["loss"].reshape(())
    if _trace:
        kernel.last_exec_ns = res.exec_time_ns
        kernel.last_results = res
    return np.asarray(total, np.float32).reshape(())

